# revision 42
# baseline (speedup 1.0000x reference)
"""AncProbsLayer Trainium2 kernel.

Computes anc[b, l, k*26+c] = P[b,k,token(b,l),c] where P[b,k] =
expm(tau_b * Q_k).

Host (tiny-parameter preprocessing, float64):
  Q_k -> symmetrized eigendecomposition, tau = softplus(tau_kernel)[idx],
  P[b,k] = U_k diag(exp(tau_b lam_k)) W_k for all b, then per-sequence
  lookup tables T[b] (26 x 208, bf16): rows 0-5 = one-hot rows for the 6
  special tokens, rows 6-25 = P rows for the 20 standard tokens.
  (The table entries ARE the output values, so bf16 tables bound the
  device error at bf16 rounding, far inside the 2e-2 gate.)

Device (the B*L*208 = 436 MB heavy lifting, pure data parallel, 64 of
512 sequences per NeuronCore):
  onehot26(token) via DVE is_equal on GpSimd, row-gather as PE matmuls
  (tile_position row groups per seq-in-group), PSUM -> SBUF bf16 copies
  split across Vector/Scalar, bf16 output DMA on both HWDGE queues
  (sync + scalar).  Output upcast bf16 -> fp32 on host.
"""

import sys
import numpy as np

for _p in ("/opt/trn_rl_repo",):
    if _p not in sys.path:
        sys.path.insert(0, _p)

import ml_dtypes
import concourse.bass as bass
import concourse.tile as tile
from concourse import mybir
from concourse.bass_utils import run_bass_kernel_spmd
from concourse.vector_clock import ScopedClock

B, L, K, NR, S = 512, 1024, 8, 512, 20
EXT = 26
ROW = K * EXT          # 208 output row width
N_CORES = 8
B_SH = B // N_CORES    # 64 sequences per core
N_GRP = B_SH // 4      # 16 groups of 4 sequences


def _patch_tile_drain():
    """This container's walrus rejects >1 sync-wait per instruction.  Split
    extra waits onto no-op instructions inserted just before, on the same
    engine (same program order on that engine => identical semantics)."""
    if getattr(tile.TileContext, "_drain_patched", False):
        return

    orig_lower = tile.TileContext._lower_ordered_insts

    def _split_lower(self, ordered):
        nc = self.nc
        for bb_name, insts in list(ordered.items()):
            new = []
            for inst in insts:
                si = getattr(inst, "sync_info", None)
                if si is not None and len(si.on_wait) > 1:
                    waits = list(si.on_wait)
                    for w in waits[:-1]:
                        nop = mybir.InstNoOp(
                            name=nc.get_next_instruction_name(),
                            ins=[], outs=[],
                            sync_info=mybir.SyncInfo(on_wait=[w], on_update=[]),
                            bass_nofuse=True,
                            engine=inst.engine,
                        )
                        new.append(nop)
                    inst.sync_info = mybir.SyncInfo(
                        on_wait=[waits[-1]], on_update=list(si.on_update)
                    )
                new.append(inst)
            ordered[bb_name] = new
        return orig_lower(self, ordered)

    tile.TileContext._lower_ordered_insts = _split_lower

    def _drain_and_barrier(self, tick_clock, wait_clock):
        nc = self.nc
        drain_inst = nc.sync.drain()
        wait_clock.add_sem_waits(
            drain_inst.ins, ScopedClock({None: tick_clock.global_clock})
        )
        si = drain_inst.ins.sync_info
        if si is not None and len(si.on_wait) > 1:
            waits = list(si.on_wait)
            drain_inst.ins.sync_info = mybir.SyncInfo(
                on_wait=[waits[0]], on_update=list(si.on_update)
            )
            for w in waits[1:]:
                d2 = nc.sync.drain()
                d2.ins.sync_info = mybir.SyncInfo(on_wait=[w], on_update=[])
        nc.all_engine_barrier()
        assert self.sems is not None
        popped = nc._tile_sem_poison_stack.pop()
        assert popped is self._sem_poison
        nc.clear_and_free_semaphores(list(self.sems.allocated().values()))
        nc.all_engine_barrier()

    tile.TileContext._drain_and_barrier = _drain_and_barrier
    tile.TileContext._drain_patched = True


def _softplus(x):
    return np.log1p(np.exp(-np.abs(x))) + np.maximum(x, 0.0)


def _host_prep(tau_kernel, exchangeability_kernel, frequencies, rate_indices):
    """Build per-sequence lookup tables (B, 26, 208) bf16 in float64."""
    E = exchangeability_kernel.astype(np.float64)
    freq = frequencies.astype(np.float64)
    eye = np.eye(S)
    R = _softplus(0.5 * (E + np.swapaxes(E, -1, -2))) * (1.0 - eye)
    Q = R * freq[None, None, :]
    diag = Q.sum(-1, keepdims=True)
    Q = Q - diag * eye
    mue = (freq[None, :, None] * diag).sum(-2, keepdims=True)
    Q = Q / np.maximum(mue, 1e-16)

    d = np.sqrt(freq)
    Sym = d[None, :, None] * Q / d[None, None, :]
    Sym = 0.5 * (Sym + np.swapaxes(Sym, -1, -2))
    lam, V = np.linalg.eigh(Sym)                       # (K,S), (K,S,S)
    U = V / d[:, None][None]                           # D^-1/2 V  (K, t, i)
    W = np.swapaxes(V, -1, -2) * d[None, None, :]      # V^T D^1/2 (K, i, c)

    tau = _softplus(tau_kernel.astype(np.float64))[
        np.asarray(rate_indices, dtype=np.int64)
    ]                                                   # (B,)
    e = np.exp(tau[:, None, None] * lam[None])          # (B, K, S)
    # P[b,k,t,c] = sum_i U[k,t,i] e[b,k,i] W[k,i,c]
    P = np.einsum("kti,bki,kic->bktc", U, e, W, optimize=True)

    tbl = np.zeros((B, EXT, ROW), np.float64)
    # std token t -> table row 6+t holds P[:, k, t, :] at cols k*26..k*26+19
    tbl[:, 6:EXT, :].reshape(B, S, K, EXT)[:, :, :, :S] = P.transpose(0, 2, 1, 3)
    # special token t (20..25) -> table row t-20 is one-hot at col k*26+t
    for s_ in range(EXT - S):
        for k in range(K):
            tbl[:, s_, k * EXT + S + s_] = 1.0
    return tbl.astype(ml_dtypes.bfloat16)


def _make_in_maps(inputs, rate_indices, tau_kernel, exchangeability_kernel,
                  frequencies):
    tok = np.asarray(inputs, dtype=np.int64)
    # remap: std t -> 6+t (P rows), special t -> t-20 (one-hot rows)
    tok_r = np.where(tok < S, tok + (EXT - S), tok - S).astype(np.uint8)
    tbl = _host_prep(
        np.asarray(tau_kernel), np.asarray(exchangeability_kernel),
        np.asarray(frequencies), rate_indices,
    )
    # token one-hots in the device layout: group tile [128, 128, 8] where
    # partition 32*b4 + t is (token(seq 4g+b4, p*8+c) == t), position
    # l = p*8 + c; rows 26-31 are always zero
    tokv = tok_r.reshape(B // 4, 4, 128, 8)
    ohh = (tokv[:, :, None, :, :] ==
           np.arange(32, dtype=np.uint8)[None, None, :, None, None])
    ohh = ohh.reshape(B // 4, 128, 128 * 8).astype(ml_dtypes.float8_e4m3)
    in_maps = []
    for c in range(N_CORES):
        sl = slice(c * B_SH, (c + 1) * B_SH)
        # device SBUF image: partition 32*b4 + r (r < 26) holds table row r
        # of seq 4*g + b4, free dims (g, 208); rows 26-31 unused
        dev = np.zeros((4, 32, N_GRP, ROW), dtype=ml_dtypes.bfloat16)
        dev[:, :EXT] = tbl[sl].reshape(N_GRP, 4, EXT, ROW).transpose(1, 2, 0, 3)
        in_maps.append({
            "tbl": dev.reshape(128, N_GRP, ROW),
            "ohd": np.ascontiguousarray(ohh[c * N_GRP : (c + 1) * N_GRP]),
        })
    return in_maps


def _build_bass():
    _patch_tile_drain()
    f32, bf16, u8 = mybir.dt.float32, mybir.dt.bfloat16, mybir.dt.uint8

    nc = bass.Bass("TRN2", target_bir_lowering=False, debug=False,
                   num_devices=N_CORES)
    tbl_d = nc.declare_dram_parameter("tbl", [128, N_GRP, ROW], bf16,
                                      isOutput=False)
    ohd_d = nc.declare_dram_parameter("ohd", [N_GRP, 128, 128 * 8],
                                      mybir.dt.float8e4, isOutput=False)
    out_d = nc.declare_dram_parameter("out", [B_SH, L, ROW], bf16,
                                      isOutput=True)

    with tile.TileContext(nc) as tc:
        with (
            tc.tile_pool(name="consts", bufs=1) as consts,
            tc.tile_pool(name="ohp", bufs=4) as ohp,
            tc.tile_pool(name="stage", bufs=4) as stagep,
            tc.tile_pool(name="ps", bufs=4, space="PSUM") as psp,
        ):
            # table tiles: partition 32*b4 + r (r<26) holds table row r of
            # seq 4*g + b4, free dims (g, 208); DRAM is the same image.
            # Split so early groups don't depend on the big second DMA.
            G_A = 2
            T4a = consts.tile([128, G_A, ROW], bf16)
            T4b = consts.tile([128, N_GRP - G_A, ROW], bf16)
            nc.sync.dma_start(out=T4a[:], in_=tbl_d[:, 0:G_A, :])
            nc.sync.dma_start(out=T4b[:], in_=tbl_d[:, G_A:N_GRP, :])

            def t4_of(g):
                return (T4a, g) if g < G_A else (T4b, g - G_A)

            def load_group(g):
                # host-prepared fp8 one-hots (0/1 exact): halves both the
                # HBM read and the SBUF-write queue time vs bf16
                oh = ohp.tile([128, 128, 8], mybir.dt.float8e4, tag="oh")
                nc.gpsimd.dma_start(
                    out=oh[:].rearrange("p a b -> p (a b)"), in_=ohd_d[g, :, :],
                )
                return oh

            # one-hot prefetch: groups 0-2 queued before the warm-up
            # matmuls so the PE never waits at a group boundary
            ohs = {0: load_group(0), 1: load_group(1), 2: load_group(2)}

            # PE pre-warm: ~3us of dependency-free matmuls flips the HAM
            # clock gate to 8/8 right as the first gather matmuls arrive
            warm_in = consts.tile([128, 320], bf16)
            nc.gpsimd.memset(warm_in, 0)
            for wi in range(12):
                wps = psp.tile([128, 4, 256], f32, tag="pst")
                nc.tensor.matmul(
                    wps[:].rearrange("p a b -> p (a b)")[:, 0:320],
                    lhsT=warm_in[:, 0:128], rhs=warm_in[:],
                    start=True, stop=True,
                )

            out_ap = out_d[:, :, :]
            for j in range(1, B_SH, 2):
                g = j // 4
                if j % 4 == 1 and g + 3 < N_GRP:
                    ohs[g + 3] = load_group(g + 3)
                oh_cur = ohs[g]
                T4, gi = t4_of(g)
                if j % 4 == 1:
                    stage = stagep.tile([128, 32, ROW], bf16, tag="stage")
                soff = (j % 4 // 2) * 16
                # position l = p*8 + c so each partition's 8 output rows
                # are contiguous in DRAM.  The two seqs of a pair have
                # their matmuls interleaved (alternating PE row groups) so
                # weight loads and matmuls overlap in the array, and PSUM
                # tiles cover half a seq (2 banks) so copies trail by half
                # a seq and the 8 banks pipeline across pairs.
                for h in range(2):
                    pst = {}
                    for jj in (j - 1, j):
                        pst[jj] = psp.tile([128, 4, 256], f32, tag="pst",
                                           name=f"pst_{jj}_{h}")
                    for c4 in range(4):
                        c = 4 * h + c4
                        for jj in (j - 1, j):
                            b4 = jj % 4
                            nc.tensor.matmul(
                                pst[jj][:, c4, 0:ROW],
                                lhsT=oh_cur[
                                    b4 * 32 : b4 * 32 + EXT, :, c].squeeze(),
                                rhs=T4[b4 * 32 : b4 * 32 + EXT, gi, :],
                                start=True, stop=True,
                                tile_position=(b4 * 32, 0),
                            )
                    for jj in (j - 1, j):
                        dst = stage[:, soff + (jj % 2) * 8 + 4 * h :
                                    soff + (jj % 2) * 8 + 4 * h + 4, :]
                        # engine alternates by (seq, half) so the two
                        # copies of a half-pair run concurrently and an
                        # engine never does both halves of one seq; two
                        # flips shift DVE's slight overload onto ACT
                        if (jj + h) % 2 == 0 and not (jj % 32 == 2 and h == 0):
                            nc.vector.tensor_copy(
                                out=dst, in_=pst[jj][:, :, 0:ROW])
                        else:
                            nc.scalar.copy(out=dst, in_=pst[jj][:, :, 0:ROW])
                # issue from engines with empty queues (sync / gpsimd) so
                # the DMA's serialized copy-waits never block a copy engine;
                # 9/7 split balances bytes between the two queues (the
                # gpsimd queue also carries the one-hot loads)
                # dependency-free weight loads fill the PE idle window of
                # each pair so the HAM activity monitor keeps the PE clock
                # at 8/8 (dummy weights; every real matmul reloads its own)
                for _ in range(8):
                    nc.tensor.ldweights(weights=warm_in[0:EXT, 0:128])
                q4 = j // 4
                if q4 < 2 or q4 == 15:
                    # pair-granular DMAs at the start (queues otherwise sit
                    # idle while production ramps) and at the very end (both
                    # queues drain their tails together)
                    half = j % 4 // 2
                    deng = nc.sync if (j // 2) % 2 == 0 else nc.gpsimd
                    deng.dma_start(
                        out=bass.AP(
                            tensor=out_ap.tensor, offset=(j - 1) * L * ROW,
                            ap=[[8 * ROW, 128], [L * ROW, 2], [1, 8 * ROW]]),
                        in_=stage[:, 16 * half : 16 * half + 16, :]
                        .rearrange("p (s c) j -> p s (c j)", s=2),
                    )
                elif j % 4 == 3:
                    deng = nc.gpsimd if q4 % 2 == 1 else nc.sync
                    deng.dma_start(
                        out=bass.AP(
                            tensor=out_ap.tensor, offset=(j - 3) * L * ROW,
                            ap=[[8 * ROW, 128], [L * ROW, 4], [1, 8 * ROW]]),
                        in_=stage[:].rearrange("p (s c) j -> p s (c j)", s=4),
                    )
    return nc


_NC_CACHE = None


def kernel(inputs, rate_indices, tau_kernel, exchangeability_kernel,
           frequencies):
    global _NC_CACHE
    in_maps = _make_in_maps(inputs, rate_indices, tau_kernel,
                            exchangeability_kernel, frequencies)
    if _NC_CACHE is None:
        _NC_CACHE = _build_bass()
    nc = _NC_CACHE
    res = run_bass_kernel_spmd(nc, in_maps, core_ids=list(range(N_CORES)))
    out = np.concatenate([res.results[c]["out"] for c in range(N_CORES)],
                         axis=0)
    return out.astype(np.float32)


if __name__ == "__main__":
    rng = np.random.default_rng(0)
    ins = {
        "inputs": rng.integers(0, EXT, size=(B, L)).astype(np.int32),
        "rate_indices": rng.integers(0, NR, size=(B,)).astype(np.int32),
        "tau_kernel": rng.standard_normal(NR).astype(np.float32),
        "exchangeability_kernel": rng.standard_normal((K, S, S)).astype(np.float32),
        "frequencies": rng.uniform(0.01, 1.0, S).astype(np.float32),
    }
    o = kernel(**ins)
    print("kernel out", o.shape, o.dtype)


# revision 49
# speedup vs baseline: 1.0174x; 1.0174x over previous
"""AncProbsLayer Trainium2 kernel.

Computes anc[b, l, k*26+c] = P[b,k,token(b,l),c] where P[b,k] =
expm(tau_b * Q_k).

Host (tiny-parameter preprocessing, float64):
  Q_k -> symmetrized eigendecomposition, tau = softplus(tau_kernel)[idx],
  P[b,k] = U_k diag(exp(tau_b lam_k)) W_k for all b, then per-sequence
  lookup tables T[b] (26 x 208, bf16): rows 0-5 = one-hot rows for the 6
  special tokens, rows 6-25 = P rows for the 20 standard tokens.
  (The table entries ARE the output values, so bf16 tables bound the
  device error at bf16 rounding, far inside the 2e-2 gate.)

Device (the B*L*208 = 436 MB heavy lifting, pure data parallel, 64 of
512 sequences per NeuronCore):
  onehot26(token) via DVE is_equal on GpSimd, row-gather as PE matmuls
  (tile_position row groups per seq-in-group), PSUM -> SBUF bf16 copies
  split across Vector/Scalar, bf16 output DMA on both HWDGE queues
  (sync + scalar).  Output upcast bf16 -> fp32 on host.
"""

import sys
import numpy as np

for _p in ("/opt/trn_rl_repo",):
    if _p not in sys.path:
        sys.path.insert(0, _p)

import ml_dtypes
import concourse.bass as bass
import concourse.tile as tile
from concourse import mybir
from concourse.bass_utils import run_bass_kernel_spmd
from concourse.vector_clock import ScopedClock

B, L, K, NR, S = 512, 1024, 8, 512, 20
EXT = 26
ROW = K * EXT          # 208 output row width
N_CORES = 8
B_SH = B // N_CORES    # 64 sequences per core
N_GRP = B_SH // 4      # 16 groups of 4 sequences


def _patch_tile_drain():
    """This container's walrus rejects >1 sync-wait per instruction.  Split
    extra waits onto no-op instructions inserted just before, on the same
    engine (same program order on that engine => identical semantics)."""
    if getattr(tile.TileContext, "_drain_patched", False):
        return

    orig_lower = tile.TileContext._lower_ordered_insts

    def _split_lower(self, ordered):
        nc = self.nc
        for bb_name, insts in list(ordered.items()):
            new = []
            for inst in insts:
                si = getattr(inst, "sync_info", None)
                if si is not None and len(si.on_wait) > 1:
                    waits = list(si.on_wait)
                    for w in waits[:-1]:
                        nop = mybir.InstNoOp(
                            name=nc.get_next_instruction_name(),
                            ins=[], outs=[],
                            sync_info=mybir.SyncInfo(on_wait=[w], on_update=[]),
                            bass_nofuse=True,
                            engine=inst.engine,
                        )
                        new.append(nop)
                    inst.sync_info = mybir.SyncInfo(
                        on_wait=[waits[-1]], on_update=list(si.on_update)
                    )
                new.append(inst)
            ordered[bb_name] = new
        return orig_lower(self, ordered)

    tile.TileContext._lower_ordered_insts = _split_lower

    def _drain_and_barrier(self, tick_clock, wait_clock):
        nc = self.nc
        drain_inst = nc.sync.drain()
        wait_clock.add_sem_waits(
            drain_inst.ins, ScopedClock({None: tick_clock.global_clock})
        )
        si = drain_inst.ins.sync_info
        if si is not None and len(si.on_wait) > 1:
            waits = list(si.on_wait)
            drain_inst.ins.sync_info = mybir.SyncInfo(
                on_wait=[waits[0]], on_update=list(si.on_update)
            )
            for w in waits[1:]:
                d2 = nc.sync.drain()
                d2.ins.sync_info = mybir.SyncInfo(on_wait=[w], on_update=[])
        nc.all_engine_barrier()
        assert self.sems is not None
        popped = nc._tile_sem_poison_stack.pop()
        assert popped is self._sem_poison
        nc.clear_and_free_semaphores(list(self.sems.allocated().values()))
        nc.all_engine_barrier()

    tile.TileContext._drain_and_barrier = _drain_and_barrier
    tile.TileContext._drain_patched = True


def _softplus(x):
    return np.log1p(np.exp(-np.abs(x))) + np.maximum(x, 0.0)


def _host_prep(tau_kernel, exchangeability_kernel, frequencies, rate_indices):
    """Build per-sequence lookup tables (B, 26, 208) bf16 in float64."""
    E = exchangeability_kernel.astype(np.float64)
    freq = frequencies.astype(np.float64)
    eye = np.eye(S)
    R = _softplus(0.5 * (E + np.swapaxes(E, -1, -2))) * (1.0 - eye)
    Q = R * freq[None, None, :]
    diag = Q.sum(-1, keepdims=True)
    Q = Q - diag * eye
    mue = (freq[None, :, None] * diag).sum(-2, keepdims=True)
    Q = Q / np.maximum(mue, 1e-16)

    d = np.sqrt(freq)
    Sym = d[None, :, None] * Q / d[None, None, :]
    Sym = 0.5 * (Sym + np.swapaxes(Sym, -1, -2))
    lam, V = np.linalg.eigh(Sym)                       # (K,S), (K,S,S)
    U = V / d[:, None][None]                           # D^-1/2 V  (K, t, i)
    W = np.swapaxes(V, -1, -2) * d[None, None, :]      # V^T D^1/2 (K, i, c)

    tau = _softplus(tau_kernel.astype(np.float64))[
        np.asarray(rate_indices, dtype=np.int64)
    ]                                                   # (B,)
    e = np.exp(tau[:, None, None] * lam[None])          # (B, K, S)
    # P[b,k,t,c] = sum_i U[k,t,i] e[b,k,i] W[k,i,c]
    P = np.einsum("kti,bki,kic->bktc", U, e, W, optimize=True)

    tbl = np.zeros((B, EXT, ROW), np.float64)
    # std token t -> table row 6+t holds P[:, k, t, :] at cols k*26..k*26+19
    tbl[:, 6:EXT, :].reshape(B, S, K, EXT)[:, :, :, :S] = P.transpose(0, 2, 1, 3)
    # special token t (20..25) -> table row t-20 is one-hot at col k*26+t
    for s_ in range(EXT - S):
        for k in range(K):
            tbl[:, s_, k * EXT + S + s_] = 1.0
    return tbl.astype(ml_dtypes.bfloat16)


def _make_in_maps(inputs, rate_indices, tau_kernel, exchangeability_kernel,
                  frequencies):
    tok = np.asarray(inputs, dtype=np.int64)
    # remap: std t -> 6+t (P rows), special t -> t-20 (one-hot rows)
    tok_r = np.where(tok < S, tok + (EXT - S), tok - S).astype(np.uint8)
    tbl = _host_prep(
        np.asarray(tau_kernel), np.asarray(exchangeability_kernel),
        np.asarray(frequencies), rate_indices,
    )
    # token one-hots in the device layout: group tile [128, 128, 8] where
    # partition 32*b4 + t is (token(seq 4g+b4, p*8+c) == t), position
    # l = p*8 + c; rows 26-31 are always zero
    tokv = tok_r.reshape(B // 4, 4, 128, 8)
    ohh = (tokv[:, :, None, :, :] ==
           np.arange(32, dtype=np.uint8)[None, None, :, None, None])
    ohh = ohh.reshape(B // 4, 128, 128 * 8).astype(ml_dtypes.float8_e4m3)
    in_maps = []
    for c in range(N_CORES):
        sl = slice(c * B_SH, (c + 1) * B_SH)
        # device SBUF image: partition 32*b4 + r (r < 26) holds table row r
        # of seq 4*g + b4, free dims (g, 208); rows 26-31 unused
        dev = np.zeros((4, 32, N_GRP, ROW), dtype=ml_dtypes.bfloat16)
        dev[:, :EXT] = tbl[sl].reshape(N_GRP, 4, EXT, ROW).transpose(1, 2, 0, 3)
        in_maps.append({
            "tbl": dev.reshape(128, N_GRP, ROW),
            "ohd": np.ascontiguousarray(ohh[c * N_GRP : (c + 1) * N_GRP]),
        })
    return in_maps


def _build_bass():
    _patch_tile_drain()
    f32, bf16, u8 = mybir.dt.float32, mybir.dt.bfloat16, mybir.dt.uint8

    nc = bass.Bass("TRN2", target_bir_lowering=False, debug=False,
                   num_devices=N_CORES)
    tbl_d = nc.declare_dram_parameter("tbl", [128, N_GRP, ROW], bf16,
                                      isOutput=False)
    ohd_d = nc.declare_dram_parameter("ohd", [N_GRP, 128, 128 * 8],
                                      mybir.dt.float8e4, isOutput=False)
    out_d = nc.declare_dram_parameter("out", [B_SH, L, ROW], bf16,
                                      isOutput=True)

    with tile.TileContext(nc) as tc:
        with (
            tc.tile_pool(name="consts", bufs=1) as consts,
            tc.tile_pool(name="ohp", bufs=N_GRP) as ohp,
            tc.tile_pool(name="stage", bufs=4) as stagep,
            tc.tile_pool(name="ps", bufs=4, space="PSUM") as psp,
        ):
            # table tiles: partition 32*b4 + r (r<26) holds table row r of
            # seq 4*g + b4, free dims (g, 208); DRAM is the same image.
            # Split so early groups don't depend on the big second DMA.
            G_A = 2
            T4a = consts.tile([128, G_A, ROW], bf16)
            T4b = consts.tile([128, N_GRP - G_A, ROW], bf16)
            nc.sync.dma_start(out=T4a[:], in_=tbl_d[:, 0:G_A, :])
            nc.sync.dma_start(out=T4b[:], in_=tbl_d[:, G_A:N_GRP, :])

            def t4_of(g):
                return (T4a, g) if g < G_A else (T4b, g - G_A)

            def load_group(g):
                # host-prepared fp8 one-hots (0/1 exact): halves both the
                # HBM read and the SBUF-write queue time vs bf16
                oh = ohp.tile([128, 128, 8], mybir.dt.float8e4, tag="oh")
                nc.gpsimd.dma_start(
                    out=oh[:].rearrange("p a b -> p (a b)"), in_=ohd_d[g, :, :],
                )
                return oh

            # all one-hots load up front (16 KB/partition in fp8): the reads
            # ride the otherwise-idle ramp-up window, leaving the steady
            # state write stream the full HBM bandwidth
            ohs = {g: load_group(g) for g in range(N_GRP)}

            # PE pre-warm: ~3us of dependency-free matmuls flips the HAM
            # clock gate to 8/8 right as the first gather matmuls arrive
            warm_in = consts.tile([128, 320], bf16)
            nc.gpsimd.memset(warm_in, 0)
            for wi in range(12):
                wps = psp.tile([128, 4, 256], f32, tag="pst")
                nc.tensor.matmul(
                    wps[:].rearrange("p a b -> p (a b)")[:, 0:320],
                    lhsT=warm_in[:, 0:128], rhs=warm_in[:],
                    start=True, stop=True,
                )

            out_ap = out_d[:, :, :]
            for j in range(1, B_SH, 2):
                g = j // 4
                oh_cur = ohs[g]
                T4, gi = t4_of(g)
                if j % 4 == 1:
                    stage = stagep.tile([128, 32, ROW], bf16, tag="stage")
                soff = (j % 4 // 2) * 16
                # position l = p*8 + c so each partition's 8 output rows
                # are contiguous in DRAM.  The two seqs of a pair have
                # their matmuls interleaved (alternating PE row groups) so
                # weight loads and matmuls overlap in the array, and PSUM
                # tiles cover half a seq (2 banks) so copies trail by half
                # a seq and the 8 banks pipeline across pairs.
                for h in range(2):
                    pst = {}
                    for jj in (j - 1, j):
                        pst[jj] = psp.tile([128, 4, 256], f32, tag="pst",
                                           name=f"pst_{jj}_{h}")
                    for c4 in range(4):
                        c = 4 * h + c4
                        for jj in (j - 1, j):
                            b4 = jj % 4
                            nc.tensor.matmul(
                                pst[jj][:, c4, 0:ROW],
                                lhsT=oh_cur[
                                    b4 * 32 : b4 * 32 + EXT, :, c].squeeze(),
                                rhs=T4[b4 * 32 : b4 * 32 + EXT, gi, :],
                                start=True, stop=True,
                                tile_position=(b4 * 32, 0),
                            )
                    for jj in (j - 1, j):
                        dst = stage[:, soff + (jj % 2) * 8 + 4 * h :
                                    soff + (jj % 2) * 8 + 4 * h + 4, :]
                        # engine alternates by (seq, half) so the two
                        # copies of a half-pair run concurrently and an
                        # engine never does both halves of one seq; two
                        # flips shift DVE's slight overload onto ACT
                        if (jj + h) % 2 == 0 and not (jj % 32 == 2 and h == 0):
                            nc.vector.tensor_copy(
                                out=dst, in_=pst[jj][:, :, 0:ROW])
                        else:
                            nc.scalar.copy(out=dst, in_=pst[jj][:, :, 0:ROW])
                # issue from engines with empty queues (sync / gpsimd) so
                # the DMA's serialized copy-waits never block a copy engine;
                # 9/7 split balances bytes between the two queues (the
                # gpsimd queue also carries the one-hot loads)
                q4 = j // 4
                if q4 < 2 or q4 == 15:
                    # pair-granular DMAs at the start (the sync queue is
                    # otherwise idle while production ramps; the gpsimd
                    # queue is busy with one-hot loads then) and at the
                    # very end (both queues drain their tails together)
                    half = j % 4 // 2
                    deng = nc.gpsimd if q4 == 15 and half == 1 else nc.sync
                    deng.dma_start(
                        out=bass.AP(
                            tensor=out_ap.tensor, offset=(j - 1) * L * ROW,
                            ap=[[8 * ROW, 128], [L * ROW, 2], [1, 8 * ROW]]),
                        in_=stage[:, 16 * half : 16 * half + 16, :]
                        .rearrange("p (s c) j -> p s (c j)", s=2),
                    )
                elif j % 4 == 3:
                    deng = nc.gpsimd if q4 % 2 == 0 else nc.sync
                    deng.dma_start(
                        out=bass.AP(
                            tensor=out_ap.tensor, offset=(j - 3) * L * ROW,
                            ap=[[8 * ROW, 128], [L * ROW, 4], [1, 8 * ROW]]),
                        in_=stage[:].rearrange("p (s c) j -> p s (c j)", s=4),
                    )
    return nc


_NC_CACHE = None


def kernel(inputs, rate_indices, tau_kernel, exchangeability_kernel,
           frequencies):
    global _NC_CACHE
    in_maps = _make_in_maps(inputs, rate_indices, tau_kernel,
                            exchangeability_kernel, frequencies)
    if _NC_CACHE is None:
        _NC_CACHE = _build_bass()
    nc = _NC_CACHE
    res = run_bass_kernel_spmd(nc, in_maps, core_ids=list(range(N_CORES)))
    out = np.concatenate([res.results[c]["out"] for c in range(N_CORES)],
                         axis=0)
    return out.astype(np.float32)


if __name__ == "__main__":
    rng = np.random.default_rng(0)
    ins = {
        "inputs": rng.integers(0, EXT, size=(B, L)).astype(np.int32),
        "rate_indices": rng.integers(0, NR, size=(B,)).astype(np.int32),
        "tau_kernel": rng.standard_normal(NR).astype(np.float32),
        "exchangeability_kernel": rng.standard_normal((K, S, S)).astype(np.float32),
        "frequencies": rng.uniform(0.01, 1.0, S).astype(np.float32),
    }
    o = kernel(**ins)
    print("kernel out", o.shape, o.dtype)


# revision 54
# speedup vs baseline: 1.2493x; 1.2279x over previous
"""AncProbsLayer Trainium2 kernel.

Computes anc[b, l, k*26+c] = P[b,k,token(b,l),c] where P[b,k] =
expm(tau_b * Q_k).

Host (tiny-parameter preprocessing, float64):
  Q_k -> symmetrized eigendecomposition, tau = softplus(tau_kernel)[idx],
  P[b,k] = U_k diag(exp(tau_b lam_k)) W_k for all b, then per-sequence
  lookup tables T[b] (26 x 208, bf16): rows 0-5 = one-hot rows for the 6
  special tokens, rows 6-25 = P rows for the 20 standard tokens.
  (The table entries ARE the output values, so bf16 tables bound the
  device error at bf16 rounding, far inside the 2e-2 gate.)

Device (the B*L*208 = 436 MB heavy lifting, pure data parallel, 64 of
512 sequences per NeuronCore):
  onehot26(token) via DVE is_equal on GpSimd, row-gather as PE matmuls
  (tile_position row groups per seq-in-group), PSUM -> SBUF bf16 copies
  split across Vector/Scalar, bf16 output DMA on both HWDGE queues
  (sync + scalar).  Output upcast bf16 -> fp32 on host.
"""

import sys
import numpy as np

for _p in ("/opt/trn_rl_repo",):
    if _p not in sys.path:
        sys.path.insert(0, _p)

import ml_dtypes
import concourse.bass as bass
import concourse.tile as tile
from concourse import mybir
from concourse.bass_utils import run_bass_kernel_spmd
from concourse.vector_clock import ScopedClock

B, L, K, NR, S = 512, 1024, 8, 512, 20
EXT = 26
ROW = K * EXT          # 208 output row width
N_CORES = 8
B_SH = B // N_CORES    # 64 sequences per core
N_GRP = B_SH // 4      # 16 groups of 4 sequences


def _patch_tile_drain():
    """This container's walrus rejects >1 sync-wait per instruction.  Split
    extra waits onto no-op instructions inserted just before, on the same
    engine (same program order on that engine => identical semantics)."""
    if getattr(tile.TileContext, "_drain_patched", False):
        return

    orig_lower = tile.TileContext._lower_ordered_insts

    def _split_lower(self, ordered):
        nc = self.nc
        for bb_name, insts in list(ordered.items()):
            new = []
            for inst in insts:
                si = getattr(inst, "sync_info", None)
                if si is not None and len(si.on_wait) > 1:
                    waits = list(si.on_wait)
                    for w in waits[:-1]:
                        nop = mybir.InstNoOp(
                            name=nc.get_next_instruction_name(),
                            ins=[], outs=[],
                            sync_info=mybir.SyncInfo(on_wait=[w], on_update=[]),
                            bass_nofuse=True,
                            engine=inst.engine,
                        )
                        new.append(nop)
                    inst.sync_info = mybir.SyncInfo(
                        on_wait=[waits[-1]], on_update=list(si.on_update)
                    )
                new.append(inst)
            ordered[bb_name] = new
        return orig_lower(self, ordered)

    tile.TileContext._lower_ordered_insts = _split_lower

    def _drain_and_barrier(self, tick_clock, wait_clock):
        nc = self.nc
        drain_inst = nc.sync.drain()
        wait_clock.add_sem_waits(
            drain_inst.ins, ScopedClock({None: tick_clock.global_clock})
        )
        si = drain_inst.ins.sync_info
        if si is not None and len(si.on_wait) > 1:
            waits = list(si.on_wait)
            drain_inst.ins.sync_info = mybir.SyncInfo(
                on_wait=[waits[0]], on_update=list(si.on_update)
            )
            for w in waits[1:]:
                d2 = nc.sync.drain()
                d2.ins.sync_info = mybir.SyncInfo(on_wait=[w], on_update=[])
        nc.all_engine_barrier()
        assert self.sems is not None
        popped = nc._tile_sem_poison_stack.pop()
        assert popped is self._sem_poison
        nc.clear_and_free_semaphores(list(self.sems.allocated().values()))
        nc.all_engine_barrier()

    tile.TileContext._drain_and_barrier = _drain_and_barrier
    tile.TileContext._drain_patched = True


def _softplus(x):
    return np.log1p(np.exp(-np.abs(x))) + np.maximum(x, 0.0)


def _host_prep(tau_kernel, exchangeability_kernel, frequencies, rate_indices):
    """Build per-sequence lookup tables (B, 26, 208) bf16 in float64."""
    E = exchangeability_kernel.astype(np.float64)
    freq = frequencies.astype(np.float64)
    eye = np.eye(S)
    R = _softplus(0.5 * (E + np.swapaxes(E, -1, -2))) * (1.0 - eye)
    Q = R * freq[None, None, :]
    diag = Q.sum(-1, keepdims=True)
    Q = Q - diag * eye
    mue = (freq[None, :, None] * diag).sum(-2, keepdims=True)
    Q = Q / np.maximum(mue, 1e-16)

    d = np.sqrt(freq)
    Sym = d[None, :, None] * Q / d[None, None, :]
    Sym = 0.5 * (Sym + np.swapaxes(Sym, -1, -2))
    lam, V = np.linalg.eigh(Sym)                       # (K,S), (K,S,S)
    U = V / d[:, None][None]                           # D^-1/2 V  (K, t, i)
    W = np.swapaxes(V, -1, -2) * d[None, None, :]      # V^T D^1/2 (K, i, c)

    tau = _softplus(tau_kernel.astype(np.float64))[
        np.asarray(rate_indices, dtype=np.int64)
    ]                                                   # (B,)
    e = np.exp(tau[:, None, None] * lam[None])          # (B, K, S)
    # P[b,k,t,c] = sum_i U[k,t,i] e[b,k,i] W[k,i,c]
    P = np.einsum("kti,bki,kic->bktc", U, e, W, optimize=True)

    tbl = np.zeros((B, EXT, ROW), np.float64)
    # std token t -> table row 6+t holds P[:, k, t, :] at cols k*26..k*26+19
    tbl[:, 6:EXT, :].reshape(B, S, K, EXT)[:, :, :, :S] = P.transpose(0, 2, 1, 3)
    # special token t (20..25) -> table row t-20 is one-hot at col k*26+t
    for s_ in range(EXT - S):
        for k in range(K):
            tbl[:, s_, k * EXT + S + s_] = 1.0
    return tbl.astype(ml_dtypes.bfloat16)


def _make_in_maps(inputs, rate_indices, tau_kernel, exchangeability_kernel,
                  frequencies):
    tok = np.asarray(inputs, dtype=np.int64)
    # remap: std t -> 6+t (P rows), special t -> t-20 (one-hot rows)
    tok_r = np.where(tok < S, tok + (EXT - S), tok - S).astype(np.uint8)
    tbl = _host_prep(
        np.asarray(tau_kernel), np.asarray(exchangeability_kernel),
        np.asarray(frequencies), rate_indices,
    )
    # token one-hots in the device layout: group tile [128, 128, 8] where
    # partition 32*b4 + t is (token(seq 4g+b4, p*8+c) == t), position
    # l = p*8 + c; rows 26-31 are always zero
    tokv = tok_r.reshape(B // 4, 4, 128, 8)
    ohh = (tokv[:, :, None, :, :] ==
           np.arange(32, dtype=np.uint8)[None, None, :, None, None])
    ohh = ohh.reshape(B // 4, 128, 128 * 8).astype(ml_dtypes.float8_e4m3)
    in_maps = []
    for c in range(N_CORES):
        sl = slice(c * B_SH, (c + 1) * B_SH)
        # device SBUF image: partition 32*b4 + r (r < 26) holds table row r
        # of seq 4*g + b4, free dims (g, 208); rows 26-31 unused
        dev = np.zeros((4, 32, N_GRP, ROW), dtype=ml_dtypes.bfloat16)
        dev[:, :EXT] = tbl[sl].reshape(N_GRP, 4, EXT, ROW).transpose(1, 2, 0, 3)
        in_maps.append({
            "tbl": dev.reshape(128, N_GRP, ROW),
            "ohd": np.ascontiguousarray(ohh[c * N_GRP : (c + 1) * N_GRP]),
        })
    return in_maps


def _build_bass():
    _patch_tile_drain()
    f32, bf16, u8 = mybir.dt.float32, mybir.dt.bfloat16, mybir.dt.uint8

    nc = bass.Bass("TRN2", target_bir_lowering=False, debug=False,
                   num_devices=N_CORES)
    tbl_d = nc.declare_dram_parameter("tbl", [128, N_GRP, ROW], bf16,
                                      isOutput=False)
    ohd_d = nc.declare_dram_parameter("ohd", [N_GRP, 128, 128 * 8],
                                      mybir.dt.float8e4, isOutput=False)
    out_d = nc.declare_dram_parameter("out", [B_SH, L, ROW], bf16,
                                      isOutput=True)

    with tile.TileContext(nc) as tc:
        with (
            tc.tile_pool(name="consts", bufs=1) as consts,
            tc.tile_pool(name="ohp", bufs=4) as ohp,
            tc.tile_pool(name="stage", bufs=4) as stagep,
            tc.tile_pool(name="ps", bufs=4, space="PSUM") as psp,
        ):
            # table tiles: partition 32*b4 + r (r<26) holds table row r of
            # seq 4*g + b4, free dims (g, 208); DRAM is the same image.
            # Split so early groups don't depend on the big second DMA.
            G_A = 2
            T4a = consts.tile([128, G_A, ROW], bf16)
            T4b = consts.tile([128, N_GRP - G_A, ROW], bf16)
            nc.sync.dma_start(out=T4a[:], in_=tbl_d[:, 0:G_A, :])
            nc.sync.dma_start(out=T4b[:], in_=tbl_d[:, G_A:N_GRP, :])

            def t4_of(g):
                return (T4a, g) if g < G_A else (T4b, g - G_A)

            def load_group(g):
                # host-prepared fp8 one-hots (0/1 exact): halves both the
                # HBM read and the SBUF-write queue time vs bf16
                oh = ohp.tile([128, 128, 8], mybir.dt.float8e4, tag="oh")
                nc.gpsimd.dma_start(
                    out=oh[:].rearrange("p a b -> p (a b)"), in_=ohd_d[g, :, :],
                )
                return oh

            # one-hot prefetch: groups 0-2 queued before the warm-up
            # matmuls so the PE never waits at a group boundary
            ohs = {0: load_group(0), 1: load_group(1), 2: load_group(2)}

            # PE pre-warm: ~3us of dependency-free matmuls flips the HAM
            # clock gate to 8/8 right as the first gather matmuls arrive
            warm_in = consts.tile([128, 320], bf16)
            nc.gpsimd.memset(warm_in, 0)
            for wi in range(12):
                wps = psp.tile([128, 4, 256], f32, tag="pst")
                nc.tensor.matmul(
                    wps[:].rearrange("p a b -> p (a b)")[:, 0:320],
                    lhsT=warm_in[:, 0:128], rhs=warm_in[:],
                    start=True, stop=True,
                )

            out_ap = out_d[:, :, :]
            for j in range(1, B_SH, 2):
                g = j // 4
                if j % 4 == 1 and g + 3 < N_GRP:
                    ohs[g + 3] = load_group(g + 3)
                oh_cur = ohs[g]
                T4, gi = t4_of(g)
                if j % 4 == 1:
                    stage = stagep.tile([128, 32, ROW], bf16, tag="stage")
                soff = (j % 4 // 2) * 16
                # position l = p*8 + c so each partition's 8 output rows
                # are contiguous in DRAM.  The two seqs of a pair have
                # their matmuls interleaved (alternating PE row groups) so
                # weight loads and matmuls overlap in the array, and PSUM
                # tiles cover half a seq (2 banks) so copies trail by half
                # a seq and the 8 banks pipeline across pairs.
                for h in range(2):
                    pst = {}
                    for jj in (j - 1, j):
                        pst[jj] = psp.tile([128, 4, 256], f32, tag="pst",
                                           name=f"pst_{jj}_{h}")
                    for c4 in range(4):
                        c = 4 * h + c4
                        for jj in (j - 1, j):
                            b4 = jj % 4
                            nc.tensor.matmul(
                                pst[jj][:, c4, 0:ROW],
                                lhsT=oh_cur[
                                    b4 * 32 : b4 * 32 + EXT, :, c].squeeze(),
                                rhs=T4[b4 * 32 : b4 * 32 + EXT, gi, :],
                                start=True, stop=True,
                                tile_position=(b4 * 32, 0),
                            )
                    for jj in (j - 1, j):
                        dst = stage[:, soff + (jj % 2) * 8 + 4 * h :
                                    soff + (jj % 2) * 8 + 4 * h + 4, :]
                        # engine alternates by (seq, half) so the two
                        # copies of a half-pair run concurrently and an
                        # engine never does both halves of one seq; two
                        # flips shift DVE's slight overload onto ACT
                        if (jj + h) % 2 == 0 and not (jj % 32 == 2 and h == 0):
                            nc.vector.tensor_copy(
                                out=dst, in_=pst[jj][:, :, 0:ROW])
                        else:
                            nc.scalar.copy(out=dst, in_=pst[jj][:, :, 0:ROW])
                # issue from engines with empty queues (sync / gpsimd) so
                # the DMA's serialized copy-waits never block a copy engine;
                # 9/7 split balances bytes between the two queues (the
                # gpsimd queue also carries the one-hot loads)
                q4 = j // 4
                if q4 < 2 or q4 == 15:
                    # pair-granular DMAs at the start (the sync queue is
                    # otherwise idle while production ramps; the gpsimd
                    # queue is busy with one-hot loads then) and at the
                    # very end (both queues drain their tails together)
                    half = j % 4 // 2
                    deng = nc.sync if (j // 2) % 2 == 0 else nc.gpsimd
                    deng.dma_start(
                        out=bass.AP(
                            tensor=out_ap.tensor, offset=(j - 1) * L * ROW,
                            ap=[[8 * ROW, 128], [L * ROW, 2], [1, 8 * ROW]]),
                        in_=stage[:, 16 * half : 16 * half + 16, :]
                        .rearrange("p (s c) j -> p s (c j)", s=2),
                    )
                elif j % 4 == 3:
                    deng = nc.gpsimd if q4 % 2 == 1 else nc.sync
                    deng.dma_start(
                        out=bass.AP(
                            tensor=out_ap.tensor, offset=(j - 3) * L * ROW,
                            ap=[[8 * ROW, 128], [L * ROW, 4], [1, 8 * ROW]]),
                        in_=stage[:].rearrange("p (s c) j -> p s (c j)", s=4),
                    )
    return nc


_NC_CACHE = None


def kernel(inputs, rate_indices, tau_kernel, exchangeability_kernel,
           frequencies):
    global _NC_CACHE
    in_maps = _make_in_maps(inputs, rate_indices, tau_kernel,
                            exchangeability_kernel, frequencies)
    if _NC_CACHE is None:
        _NC_CACHE = _build_bass()
    nc = _NC_CACHE
    res = run_bass_kernel_spmd(nc, in_maps, core_ids=list(range(N_CORES)))
    out = np.concatenate([res.results[c]["out"] for c in range(N_CORES)],
                         axis=0)
    return out.astype(np.float32)


if __name__ == "__main__":
    rng = np.random.default_rng(0)
    ins = {
        "inputs": rng.integers(0, EXT, size=(B, L)).astype(np.int32),
        "rate_indices": rng.integers(0, NR, size=(B,)).astype(np.int32),
        "tau_kernel": rng.standard_normal(NR).astype(np.float32),
        "exchangeability_kernel": rng.standard_normal((K, S, S)).astype(np.float32),
        "frequencies": rng.uniform(0.01, 1.0, S).astype(np.float32),
    }
    o = kernel(**ins)
    print("kernel out", o.shape, o.dtype)
